# revision 34
# baseline (speedup 1.0000x reference)
"""Fused multi-head self-attention (concat-head, scale=sqrt(d_model)) on 8 trn2 cores.

Sharding: batch(4) x key-half(2) -> 8 cores. Each core:
  - input xT = x[b].T [F=512, T=2048], pre-transposed AND column-rotated on
    host so that this core's key-half is always columns 0:1024 (the rotation
    keeps the device program identical across cores; host un-rolls outputs).
  - host fuses M = Wq @ Wk^T (fp64, exact), so scores = Xq M Xkv^T needs a
    single projection yT = M^T @ xqT instead of separate q/k projections
  - computes yT (all queries), v = xkvT^T @ Wv (its 1024 keys only)
  - scoresT[s, tq] = xkvT_slice.T @ yT (contract feature dim), then
    expT = exp(scoresT / sqrt(512)) (no max-subtraction needed: scores O(1))
  - partial out[tq, p] = expT.T @ v and partial row-sums (ones-vector matmul),
    both returned unnormalized; host combines the two key-halves:
    out = (o0 + o1) / (s0 + s1).
Projections/out matmuls are bf16 (fp32 accumulate); the scores matmul runs
fp8e4m3 DoubleRow (K=256/instr, ~1.8x bf16): xkv keys quantized on host,
y quantized during the PSUM->SBUF copy, scales divided back out in the exp.
"""

import os
from contextlib import ExitStack

import numpy as np
import ml_dtypes

import concourse.bass as bass
import concourse.tile as tile
import concourse.mybir as mybir
from concourse import bacc
from concourse.bass_utils import run_bass_kernel_spmd

B, T, F, P = 4, 2048, 512, 512
NCORES = 8
KSPLIT = NCORES // B          # key-dim split per batch
TKV = T // KSPLIT             # 1024 keys per core
SCALE = 1.0 / float(np.sqrt(512.0))

FT = F // 128     # 4 f-tiles (contraction of projections)
PT = P // 128     # 4 p-tiles (contraction of scores)
ST = TKV // 128   # 8 s-tiles (keys per core)
NCH = T // 512    # 4 query chunks of 512
F32 = mybir.dt.float32
FP8 = mybir.dt.float8e4
NP_FP8 = np.dtype(ml_dtypes.float8_e4m3)

# fp8 scores path: xkv and y quantized to e4m3, scores via DoubleRow
# matmuls (K=256 per instruction). Scales keep values out of the fp8
# denormal range; both divide back out inside the exp's scale factor.
YS = 32.0         # y pre-scale (folded into wm on host)
XS = 8.0          # xkv fp8 pre-scale (applied on host)
ESCALE = SCALE / (YS * XS)
# exp-minus-one fp8 output path: out = sum_s e_s v_s is split as
# colsum(v) + sum_s (e_s - 1) v_s; the colsum is added exactly on the host,
# the (e-1) part runs as fp8 DoubleRow matmuls. Quantization of (e-1) and v
# only touches the small correction term, so fp8 noise there is ~5x cheaper
# than quantizing e directly.
ES = 8.0          # (e-1) fp8 pre-scale (applied by the DVE pass)
VS = 16.0         # v fp8 pre-scale (folded into wv on host)

# matmul dtype: "bf16" (1 cyc/row) | "fp32" (4 cyc/row, exact)
KDT = os.environ.get("KDT", "bf16")


def _mm_dtypes():
    if KDT == "bf16":
        return mybir.dt.bfloat16, np.dtype(ml_dtypes.bfloat16)
    elif KDT == "fp32":
        return mybir.dt.float32, np.dtype(np.float32)
    else:
        raise ValueError(KDT)


def _attn_body(ctx, tc, xqt, xq8, wm, wv, out, sums):
    nc = tc.nc
    DT, _ = _mm_dtypes()
    DR = mybir.MatmulPerfMode.DoubleRow
    Exp = mybir.ActivationFunctionType.Exp

    consts = ctx.enter_context(tc.tile_pool(name="consts", bufs=1))
    persist = ctx.enter_context(tc.tile_pool(name="persist", bufs=1))
    exp_pool = ctx.enter_context(tc.tile_pool(name="expp", bufs=2))
    out_pool = ctx.enter_context(tc.tile_pool(name="outsb", bufs=6))
    small = ctx.enter_context(tc.tile_pool(name="small", bufs=2))
    ps_sc = ctx.enter_context(tc.tile_pool(name="pssc", bufs=3, space="PSUM"))
    ps_out = ctx.enter_context(tc.tile_pool(name="psout", bufs=4, space="PSUM"))
    ps_sum = ctx.enter_context(tc.tile_pool(name="pssum", bufs=1, space="PSUM"))

    # ---- PE warmup: junk matmuls with no DMA deps, overlap the HAM ramp
    # and the initial input DMAs. All junk targets ONE psum tile (WAW on the
    # same engine is program-ordered -> no cross-engine sems, no pool DRAINs)
    junk = consts.tile([128, 128], DT, tag="junk", name="junk")
    nc.vector.memset(junk, 0.0)
    # junk psum lives in the out0 bank, idle until chunk 0's out matmuls
    # (which follow all junk in PE program order -> no cross-engine sems)
    wu_ps = ps_out.tile([128, 512], F32, tag="out0", name="wu_ps", bufs=1)
    for w in range(52):
        nc.tensor.matmul(
            wu_ps[:, 0:128], junk, junk, start=True, stop=True,
            skip_group_check=True,
        )

    # ---- load weights + inputs (wq/xq chunk 0 first so qT starts early) ----
    wm_sb = [consts.tile([128, P], DT, tag=f"wm{i}", name=f"wm{i}") for i in range(FT)]
    wv_sb = [consts.tile([128, P], DT, tag=f"wv{i}", name=f"wv{i}") for i in range(FT)]
    xq_sb = [consts.tile([128, T], DT, tag=f"xq{i}", name=f"xq{i}") for i in range(FT)]
    # v/kT inputs first (smallest working set), need-ordered round-robin on
    # all three DMA queues; qT inputs follow (needed only ~24us in)
    qS, qG, qA = nc.sync, nc.gpsimd, nc.scalar

    # fp8 copy of this core's keys, DoubleRow pair layout: tile j slot i
    # holds f-rows j*256+i*128 .. +128
    xq8_sb = [
        consts.tile([128, 2, TKV], FP8, tag=f"xq8_{j}", name=f"xq8_{j}")
        for j in range(2)
    ]

    # priority order: y-proj c0 inputs (wm + xq chunk 0), then scores c0
    # (xq8), then v-proj (wv + xq chunk 1), then the late query columns.
    # Queue plan: each engine queue runs ~5 dma_starts back-to-back, then
    # ring-depth throttling serializes on completions -- so scalar (exp) and
    # vector (casts) only take EARLY transfers that finish before their
    # engine duties begin; sync/gpsimd absorb the throttled late ones.
    q8 = [qS, qG, qA, qS, qG, qA, qS, qG]
    for i in range(FT):
        q8[2 * i].dma_start(out=wm_sb[i], in_=wm[i * 128 : (i + 1) * 128, :])
        q8[2 * i + 1].dma_start(
            out=xq_sb[i][:, 0:512], in_=xqt[i * 128 : (i + 1) * 128, 0:512]
        )
    q4 = [qA, qS, qG, qA]
    for j in range(2):
        for i in range(2):
            q4[2 * j + i].dma_start(
                out=xq8_sb[j][:, i, :],
                in_=xq8[(2 * j + i) * 128 : (2 * j + i + 1) * 128, :],
            )
    qL = [qS, qG, qS, qG]
    for i in range(FT):
        qL[i].dma_start(out=wv_sb[i], in_=wv[i * 128 : (i + 1) * 128, :])
    for i in range(FT):
        qL[i].dma_start(
            out=xq_sb[i][:, 512:1024], in_=xqt[i * 128 : (i + 1) * 128, 512:1024]
        )
    for i in range(FT):
        qL[i].dma_start(
            out=xq_sb[i][:, 1024:2048], in_=xqt[i * 128 : (i + 1) * 128, 1024:2048]
        )

    # DR pair of 1.0-columns for the (e-1) row-sum matmuls; [128,2,16] so the
    # pair stride stays 16B-aligned, only column 0 is used
    ones2 = consts.tile([128, 2, 16], FP8, tag="ones2", name="ones2")
    nc.vector.memset(ones2, 1.0)

    # ---- projections, v first to match DMA arrival; yT chunks run inside
    # the attention loop where they have huge DMA slack ----
    yt_sb = [
        persist.tile([128, 2, T], FP8, tag=f"yt{m}", name=f"yt{m}")
        for m in range(2)
    ]
    v2_sb = [
        persist.tile([128, 2, P], FP8, tag=f"v2_{u}", name=f"v2_{u}")
        for u in range(ST // 2)
    ]

    def v_step(s):
        ps = ps_sc.tile([128, 512], F32, tag="sc", name="ps_v")
        for kf in range(FT):
            nc.tensor.matmul(
                ps,
                xq_sb[kf][:, s * 128 : (s + 1) * 128],
                wv_sb[kf],
                start=kf == 0,
                stop=kf == FT - 1,
            )
        nc.vector.tensor_copy(out=v2_sb[s // 2][:, s % 2, :], in_=ps)

    # ---- attention, per query chunk of 512; out-accumulation s-outer,
    # pipelined one s-step behind scores so PE never waits on ACT exp ----
    for c in range(NCH):
        qs = slice(c * 512, (c + 1) * 512)
        # yT for this query chunk only -- spreads the xq DMA need across the
        # whole kernel instead of front-loading it
        for m in range(PT - 1):
            ps = ps_sc.tile([128, 512], F32, tag="sc", name="ps_y")
            for kf in range(FT):
                nc.tensor.matmul(
                    ps,
                    wm_sb[kf][:, m * 128 : (m + 1) * 128],
                    xq_sb[kf][:, qs],
                    start=kf == 0,
                    stop=kf == FT - 1,
                )
            nc.vector.tensor_copy(out=yt_sb[m // 2][:, m % 2, qs], in_=ps)
        # last p-tile (m=3) in two column-half psum groups so its fp8 casts
        # drain earlier; the first scores matmuls of this chunk then never
        # wait on the cast
        for half in range(2):
            ph = ps_sc.tile([128, 256], F32, tag="sc", name="ps_yh")
            hs = slice(c * 512 + half * 256, c * 512 + half * 256 + 256)
            for kf in range(FT):
                nc.tensor.matmul(
                    ph,
                    wm_sb[kf][:, 384:512],
                    xq_sb[kf][:, hs],
                    start=kf == 0,
                    stop=kf == FT - 1,
                )
            nc.vector.tensor_copy(out=yt_sb[1][:, 1, hs], in_=ph)
        # bf16 exp is transient (em1 input only); e2 holds ES*(exp-1) in fp8
        # DoubleRow pair layout, alive until this chunk's out matmuls read it
        e16 = [
            exp_pool.tile([128, 512], DT, tag=f"e16_{s % 3}", name=f"e16_{s % 3}")
            for s in range(ST)
        ]
        e2 = [
            exp_pool.tile([128, 2, 512], FP8, tag=f"e2_{u}", name=f"e2_{u}")
            for u in range(ST // 2)
        ]
        sums_ps = ps_sum.tile([1, 512], F32, tag="sums", name="sums_ps")
        po = [
            ps_out.tile([128, 512], F32, tag=f"out{t4}", name=f"po{t4}", bufs=1)
            for t4 in range(4)
        ]
        oq = [nc.sync, nc.scalar, nc.gpsimd, nc.sync]

        def scores_mm(s, ps, j):
            nc.tensor.matmul(
                ps,
                xq8_sb[j][:, :, s * 128 : (s + 1) * 128],
                yt_sb[j][:, :, qs],
                start=j == 0,
                stop=j == 1,
                perf_mode=DR,
                skip_group_check=True,
            )

        def scores_act(s, ps, pieces=1):
            w = 512 // pieces
            for p in range(pieces):
                cs = slice(p * w, (p + 1) * w)
                nc.scalar.activation(
                    out=e16[s][:, cs], in_=ps[:, cs], func=Exp, scale=ESCALE
                )

        def em1_pass(s, pieces=1):
            w = 512 // pieces
            for p in range(pieces):
                cs = slice(p * w, (p + 1) * w)
                nc.vector.tensor_scalar(
                    out=e2[s // 2][:, s % 2, cs],
                    in0=e16[s][:, cs],
                    scalar1=-1.0,
                    scalar2=ES,
                    op0=mybir.AluOpType.add,
                    op1=mybir.AluOpType.mult,
                )

        def scores_step(s):
            ps = ps_sc.tile([128, 512], F32, tag="sc", name="ps_sc")
            scores_mm(s, ps, 0)
            scores_mm(s, ps, 1)
            # last chunk's final exp/em1 go in 4 column pieces so the tail
            # out-matmuls (lhsT = one 128-col piece each) start as pieces land
            pieces = 4 if (c == NCH - 1 and s == ST - 1) else 1
            scores_act(s, ps, pieces)
            em1_pass(s, pieces)

        def sums_mm(u):
            # row-sums of ES*(e-1) over the key dim, same DR pairing
            nc.tensor.matmul(
                sums_ps,
                ones2[:, :, 0:1],
                e2[u],
                start=u == 0,
                stop=u == ST // 2 - 1,
                perf_mode=DR,
                skip_group_check=True,
            )

        def pair_step(u, only_t4=None, do_sums=True):
            # sums first: its psum group then stops ~1us before the po banks,
            # so the sums evacuation chain fully drains off the critical tail
            if do_sums:
                sums_mm(u)
            rng = range(4) if only_t4 is None else (only_t4,)
            for t4 in rng:
                nc.tensor.matmul(
                    po[t4],
                    e2[u][:, :, t4 * 128 : (t4 + 1) * 128],
                    v2_sb[u],
                    start=u == 0,
                    stop=u == ST // 2 - 1,
                    perf_mode=DR,
                    skip_group_check=True,
                )

        def evac(t4):
            # mid-kernel: only t4=0 on DVE -- the next chunk's y-casts queue
            # behind these and DVE is the busier engine. Last chunk: 2/2
            # DVE/ACT split minimizes the serial tail (same-bank PSUM reads
            # can't overlap across engines; neighbors target different banks)
            tt = c * 4 + t4
            osb = out_pool.tile([128, 512], DT, tag="osb", name="osb")
            on_dve = (t4 % 2 == 0) if c == NCH - 1 else (t4 == 0)
            if on_dve:
                nc.vector.tensor_copy(out=osb, in_=po[t4])
            else:
                nc.scalar.copy(out=osb, in_=po[t4])
            oq[t4].dma_start(out=out[tt * 128 : (tt + 1) * 128, :], in_=osb)

        # interleave the first two score steps at the j level: the j=0
        # matmuls only need yt_sb[0] (cast early in y-proj), buying time
        # for the final yt_sb[1] casts to drain
        ps0 = ps_sc.tile([128, 512], F32, tag="sc", name="ps_s0")
        ps1 = ps_sc.tile([128, 512], F32, tag="sc", name="ps_s1")
        scores_mm(0, ps0, 0)
        scores_mm(1, ps1, 0)
        scores_mm(0, ps0, 1)
        scores_act(0, ps0)
        scores_mm(1, ps1, 1)
        scores_act(1, ps1)
        if c == 0:
            # v-projection rides inside chunk 0's dense pipeline (its inputs
            # arrive last); v2 casts go on DVE before the em1 passes so the
            # psum ring frees up promptly
            v_step(0)
            v_step(1)
        em1_pass(0)
        em1_pass(1)
        for s in range(2, ST):
            scores_step(s)
            if c == 0:
                v_step(s)
            if s % 2 == 1 and s >= 3:
                pair_step((s - 3) // 2)
        pair_step(ST // 2 - 1)
        sums_sb = small.tile([1, 512], F32, tag="sums_sb", name="sums_sb")
        if c == NCH - 1:
            nc.vector.tensor_copy(out=sums_sb, in_=sums_ps)
            nc.gpsimd.dma_start(out=sums[0:1, qs], in_=sums_sb)
        for t4 in range(4):
            evac(t4)
        if c < NCH - 1:
            nc.vector.tensor_copy(out=sums_sb, in_=sums_ps)
            nc.gpsimd.dma_start(out=sums[0:1, qs], in_=sums_sb)


_CACHE = {}


def _get_compiled():
    key = KDT
    if key in _CACHE:
        return _CACHE[key]
    DT, _ = _mm_dtypes()
    nc = bacc.Bacc(
        "TRN2",
        target_bir_lowering=False,
        debug=False,
        enable_asserts=False,
        num_devices=NCORES,
        num_swdge_queues=2,
    )
    xqt = nc.dram_tensor("xqt", [F, T], DT, kind="ExternalInput").ap()
    xq8 = nc.dram_tensor("xq8", [F, TKV], FP8, kind="ExternalInput").ap()
    wm = nc.dram_tensor("wm", [F, P], DT, kind="ExternalInput").ap()
    wv = nc.dram_tensor("wv", [F, P], DT, kind="ExternalInput").ap()
    out = nc.dram_tensor("out", [T, P], DT, kind="ExternalOutput").ap()
    sums = nc.dram_tensor("sums", [1, T], F32, kind="ExternalOutput").ap()
    with tile.TileContext(nc) as tc, ExitStack() as ctx:
        _attn_body(ctx, tc, xqt, xq8, wm, wv, out, sums)
    nc.compile()
    _CACHE[key] = nc
    return nc


def kernel(x, Wq, Wk, Wv, _trace=False):
    _, np_dt = _mm_dtypes()
    nc = _get_compiled()
    # fused scores weight: scores = Xq (Wq Wk^T) Xkv^T; fp64 on host, exact.
    # Pre-scaled by YS so the quantized y lands in fp8's normal range.
    wm_c = np.ascontiguousarray(
        (Wq.astype(np.float64) @ Wk.astype(np.float64).T * YS).astype(np_dt)
    )
    # v pre-scaled by VS so the fp8 v2 tiles stay in e4m3's normal range
    wv_c = np.ascontiguousarray((Wv.astype(np.float64) * VS).astype(np_dt))
    xT32 = [np.ascontiguousarray(x[b].T) for b in range(B)]
    in_maps = []
    for core in range(NCORES):
        b, h = divmod(core, KSPLIT)
        rolled = np.roll(xT32[b], -h * TKV, axis=1)
        in_maps.append(
            {
                # rotate so this core's key-half sits in columns 0:TKV
                "xqt": np.ascontiguousarray(rolled.astype(np_dt)),
                "xq8": np.ascontiguousarray(
                    (rolled[:, 0:TKV] * XS).astype(NP_FP8)
                ),
                "wm": wm_c,
                "wv": wv_c,
            }
        )
    res = run_bass_kernel_spmd(
        nc, in_maps, core_ids=list(range(NCORES)), trace=_trace
    )
    out = np.empty((B, T, P), np.float32)
    for b in range(B):
        o = np.zeros((T, P), np.float32)
        s = np.zeros(T, np.float32)
        for h in range(KSPLIT):
            r = res.results[b * KSPLIT + h]
            # un-rotate the query axis (device query j = original (j + h*TKV) % T)
            o += np.roll(r["out"].astype(np.float32), h * TKV, axis=0)
            s += np.roll(r["sums"][0], h * TKV)
        # device returns the (e-1)-weighted parts; the exact colsum(v) and the
        # key count complete out = sum_s e_s v_s and sums = sum_s e_s
        colsum = (
            x[b].astype(np.float64).sum(axis=0) @ Wv.astype(np.float64)
        ).astype(np.float32)
        out[b] = (o / (ES * VS) + colsum[None, :]) / (s / ES + float(T))[:, None]
    if _trace:
        return out, res
    return out

